# revision 1
# baseline (speedup 1.0000x reference)
"""Trainium2 Bass kernel for nn_GCAModel (2D ST-LSTM recurrence + classifier).

Strategy (per the batch-data-parallel hint + anti-diagonal wavefront):
  - Shard batch B=128 across 8 cores (16 rows each); weights replicated.
  - Within a core, process the (t, j) grid along anti-diagonals d = t + j.
    All cells on a diagonal are independent -> one fused GEMM per diagonal
    with moving dim N = n_cells * 16.
  - State h/c lives in SBUF as (128 part = H-chunk, 2 chunks, slot*16 cols),
    slot s holds row t = s-1 (slot 0 is a permanent zero guard). The GEMM for
    diagonal d reads the h_t operand at slot offset t_lo and the h_s operand
    at t_lo+1 -- the same buffer, shifted by one slot.
  - Gates computed as g.T: out (gate-chunk 128, N) = W.T chunk @ h-chunk,
    accumulating 5 K-groups in PSUM (x+bias K=4, Wth 2x128, Wsh 2x128).
  - ScalarE applies sigmoid/tanh (PSUM->SBUF), VectorE does the cell update,
    writing h/c back into the state buffer in place.
  - Mean-pool h via a wide fp32 accumulator += on each diagonal; classifier
    (+log_softmax) on device; output (16, 60) per core, concatenated on host.

Numerics: fp16 storage for x/weights/h/gates (PSUM accumulates fp32), fp32
cell state c (c grows beyond fp16 range; fp16-with-fp32-c measured 2.4e-4
rel err vs fp64 oracle on CPU).
"""
import os
os.environ.setdefault("JAX_PLATFORMS", "axon,cpu")

import numpy as np

import concourse.bass as bass
import concourse.tile as tile
from concourse import bacc, mybir
from concourse.bass_utils import run_bass_kernel_spmd

# ---------------------------------------------------------------- problem dims
T, J, B, I, H, C = 100, 25, 128, 3, 256, 60
NCORES = 8
BL = B // NCORES            # 16 batch rows per core
G5 = 5 * H                  # 1280 gate columns
SLOTS = T + 1               # +1 zero-guard slot at the front
SW = SLOTS * BL             # state width (free dim) per H-chunk
NMAX = min(T, J) * BL       # widest diagonal: 25*16 = 400

# gate order: process u first, o last (c needs i,fs,ft,u; h needs o + tanh(c))
GATES = [("u", 4 * H, "Tanh"), ("i", 0, "Sigmoid"), ("fs", H, "Sigmoid"),
         ("ft", 2 * H, "Sigmoid"), ("o", 3 * H, "Sigmoid")]

# diagonals: d = t + j
DIAGS = []
_off = 0
for _d in range(T + J - 1):
    _tlo, _thi = max(0, _d - (J - 1)), min(_d, T - 1)
    _nd = _thi - _tlo + 1
    DIAGS.append((_tlo, _nd, _off))
    _off += _nd * BL
XCOLS = _off                # 40000

# ---------------------------------------------------------------- dtype knobs
MM_DT = mybir.dt.float16      # x / W / h storage (matmul operands)
GATE_DT = mybir.dt.float16    # post-activation gates, t1, tanh(c)
C_DT = mybir.dt.float32       # cell state + c-proportional temps
MM_NP = np.float16

F32 = mybir.dt.float32


def _build_nc():
    nc = bacc.Bacc("TRN2", target_bir_lowering=False, debug=False,
                   num_devices=NCORES)
    x_d = nc.dram_tensor("xdiag", [4, XCOLS], MM_DT, kind="ExternalInput")
    wih_d = nc.dram_tensor("wih", [4, G5], MM_DT, kind="ExternalInput")
    wth_d = nc.dram_tensor("wth", [128, 2, G5], MM_DT, kind="ExternalInput")
    wsh_d = nc.dram_tensor("wsh", [128, 2, G5], MM_DT, kind="ExternalInput")
    wc_d = nc.dram_tensor("wc", [128, 2, C], F32, kind="ExternalInput")
    bc_d = nc.dram_tensor("bc", [1, C], F32, kind="ExternalInput")
    out_d = nc.dram_tensor("out", [BL, C], F32, kind="ExternalOutput")

    AF = mybir.ActivationFunctionType

    with tile.TileContext(nc) as tc:
        with tc.tile_pool(name="const", bufs=1) as const, \
             tc.tile_pool(name="state", bufs=1) as state, \
             tc.tile_pool(name="xin", bufs=6) as xin, \
             tc.tile_pool(name="gate", bufs=3) as gatep, \
             tc.tile_pool(name="work", bufs=3) as work, \
             tc.tile_pool(name="psg", bufs=3, space="PSUM") as psg, \
             tc.tile_pool(name="pscls", bufs=1, space="PSUM") as pscls:

            # ---- load constants
            wih_s = const.tile([4, G5], MM_DT)
            nc.sync.dma_start(out=wih_s, in_=wih_d[:, :])
            wth_s = const.tile([128, 2, G5], MM_DT)
            nc.sync.dma_start(out=wth_s, in_=wth_d[:, :, :])
            wsh_s = const.tile([128, 2, G5], MM_DT)
            nc.sync.dma_start(out=wsh_s, in_=wsh_d[:, :, :])
            wc_s = const.tile([128, 2, C], F32)
            nc.sync.dma_start(out=wc_s, in_=wc_d[:, :, :])
            bc_s = const.tile([1, C], F32)
            nc.sync.dma_start(out=bc_s, in_=bc_d[:, :])
            ones_s = const.tile([1, BL], F32)
            nc.vector.memset(ones_s, 1.0)

            # ---- state (slot 0 stays zero forever)
            h_st = state.tile([128, 2, SW], MM_DT)
            c_st = state.tile([128, 2, SW], C_DT)
            hsum = state.tile([128, 2, SW], F32)
            nc.vector.memset(h_st, 0.0)
            nc.gpsimd.memset(c_st, 0.0)
            nc.gpsimd.memset(hsum, 0.0)

            # ---- the wavefront
            for tlo, nd, xoff in DIAGS:
                N = nd * BL
                ht, hs = tlo * BL, (tlo + 1) * BL   # slot offsets (cols)

                xs = xin.tile([4, NMAX], MM_DT, tag="x")
                nc.sync.dma_start(out=xs[:, 0:N], in_=x_d[:, xoff:xoff + N])

                gt = {}
                for gname, gc, fn in GATES:
                    ps = psg.tile([128, 2, 512], F32, tag="ps")
                    for m in (0, 1):
                        mc = gc + m * 128
                        o = ps[:, m, 0:N]
                        nc.tensor.matmul(o, wih_s[:, mc:mc + 128], xs[:, 0:N],
                                         start=True, stop=False)
                        nc.tensor.matmul(o, wth_s[:, 0, mc:mc + 128],
                                         h_st[:, 0, ht:ht + N],
                                         start=False, stop=False)
                        nc.tensor.matmul(o, wth_s[:, 1, mc:mc + 128],
                                         h_st[:, 1, ht:ht + N],
                                         start=False, stop=False)
                        nc.tensor.matmul(o, wsh_s[:, 0, mc:mc + 128],
                                         h_st[:, 0, hs:hs + N],
                                         start=False, stop=False)
                        nc.tensor.matmul(o, wsh_s[:, 1, mc:mc + 128],
                                         h_st[:, 1, hs:hs + N],
                                         start=False, stop=True)
                    g = gatep.tile([128, 2, NMAX], GATE_DT, tag=gname)
                    nc.scalar.activation(out=g[:, :, 0:N], in_=ps[:, :, 0:N],
                                         func=getattr(AF, fn))
                    gt[gname] = g

                t1 = work.tile([128, 2, NMAX], GATE_DT, tag="t1")
                nc.vector.tensor_mul(t1[:, :, 0:N], gt["i"][:, :, 0:N],
                                     gt["u"][:, :, 0:N])
                t2 = work.tile([128, 2, NMAX], C_DT, tag="t2")
                nc.vector.tensor_mul(t2[:, :, 0:N], gt["fs"][:, :, 0:N],
                                     c_st[:, :, hs:hs + N])
                s12 = work.tile([128, 2, NMAX], C_DT, tag="s12")
                nc.vector.tensor_add(s12[:, :, 0:N], t1[:, :, 0:N],
                                     t2[:, :, 0:N])
                t3 = work.tile([128, 2, NMAX], C_DT, tag="t3")
                nc.vector.tensor_mul(t3[:, :, 0:N], gt["ft"][:, :, 0:N],
                                     c_st[:, :, ht:ht + N])
                nc.vector.tensor_add(c_st[:, :, hs:hs + N], s12[:, :, 0:N],
                                     t3[:, :, 0:N])
                tcz = work.tile([128, 2, NMAX], GATE_DT, tag="tc")
                nc.scalar.activation(out=tcz[:, :, 0:N],
                                     in_=c_st[:, :, hs:hs + N], func=AF.Tanh)
                nc.vector.tensor_mul(h_st[:, :, hs:hs + N],
                                     gt["o"][:, :, 0:N], tcz[:, :, 0:N])
                nc.vector.tensor_add(hsum[:, :, hs:hs + N],
                                     hsum[:, :, hs:hs + N],
                                     h_st[:, :, hs:hs + N])

            # ---- mean-pool: fold slots 1..100 down onto slot 1
            cur = T
            while cur > 1:
                if cur % 2 == 1:
                    last = BL + (cur - 1) * BL
                    nc.vector.tensor_add(hsum[:, :, BL:2 * BL],
                                         hsum[:, :, BL:2 * BL],
                                         hsum[:, :, last:last + BL])
                    cur -= 1
                half = cur // 2
                w = half * BL
                nc.vector.tensor_add(hsum[:, :, BL:BL + w],
                                     hsum[:, :, BL:BL + w],
                                     hsum[:, :, BL + w:BL + 2 * w])
                cur = half
            # F = hsum[:, :, BL:2*BL]  (128, 2, 16) fp32; 1/(T*J) folded into wc

            # ---- classifier logits.T? no: out (BL, C) with batch on partitions
            pc = pscls.tile([BL, 512], F32, tag="cls")
            lg = pc[:, 0:C]
            nc.tensor.matmul(lg, hsum[:, 0, BL:2 * BL], wc_s[:, 0, :],
                             start=True, stop=False)
            nc.tensor.matmul(lg, hsum[:, 1, BL:2 * BL], wc_s[:, 1, :],
                             start=False, stop=False)
            nc.tensor.matmul(lg, ones_s[:, :], bc_s[:, :],
                             start=False, stop=True)

            # ---- log_softmax over free dim
            mx = work.tile([BL, 1], F32, tag="mx")
            nc.vector.reduce_max(out=mx, in_=lg, axis=mybir.AxisListType.X)
            nmx = work.tile([BL, 1], F32, tag="nmx")
            nc.scalar.mul(out=nmx, in_=mx, mul=-1.0)
            ex = work.tile([BL, C], F32, tag="ex")
            nc.scalar.activation(out=ex, in_=lg, func=AF.Exp, bias=nmx)
            sm = work.tile([BL, 1], F32, tag="sm")
            nc.vector.reduce_sum(out=sm, in_=ex, axis=mybir.AxisListType.X)
            lse = work.tile([BL, 1], F32, tag="lse")
            nc.scalar.activation(out=lse, in_=sm, func=AF.Ln)
            tot = work.tile([BL, 1], F32, tag="tot")
            nc.vector.tensor_add(tot, mx, lse)
            res = work.tile([BL, C], F32, tag="res")
            nc.vector.tensor_scalar(out=res, in0=lg, scalar1=tot, scalar2=None,
                                    op0=mybir.AluOpType.subtract)
            nc.sync.dma_start(out=out_d[:, :], in_=res)

    nc.compile()
    return nc


_NC = None


def _get_nc():
    global _NC
    if _NC is None:
        _NC = _build_nc()
    return _NC


def _pack_inputs(data, W_ih, W_th, W_sh, b, weight_c, bias_c):
    """Host-side prep: weights in lhsT layout, x in diagonal-major order."""
    data = np.asarray(data, np.float32)
    # lhsT for the x-GEMM: (I+1, 1280) = [W_ih.T; b] (bias via ones row in x)
    wih = np.concatenate([np.asarray(W_ih, np.float32).T,
                          np.asarray(b, np.float32)[None, :]], 0).astype(MM_NP)
    # lhsT for h-GEMMs: (128, chunk, 1280)
    wth = np.asarray(W_th, np.float32).T.reshape(2, 128, G5).transpose(1, 0, 2)
    wsh = np.asarray(W_sh, np.float32).T.reshape(2, 128, G5).transpose(1, 0, 2)
    wth = np.ascontiguousarray(wth).astype(MM_NP)
    wsh = np.ascontiguousarray(wsh).astype(MM_NP)
    # classifier: fold the 1/(T*J) mean into the weights
    wc = (np.asarray(weight_c, np.float32).T / (T * J)).reshape(2, 128, C)
    wc = np.ascontiguousarray(wc.transpose(1, 0, 2), np.float32)
    bc = np.asarray(bias_c, np.float32)[None, :]

    # x in diagonal-major order: cols (cell-in-diag, batch), rows (I..., ones)
    tt = np.concatenate([np.arange(max(0, d - (J - 1)), min(d, T - 1) + 1)
                         for d in range(T + J - 1)])
    jj = np.concatenate([d - np.arange(max(0, d - (J - 1)), min(d, T - 1) + 1)
                         for d in range(T + J - 1)])
    xc = data[tt, jj]                     # (2500, B, I)
    in_maps = []
    for k in range(NCORES):
        xk = xc[:, k * BL:(k + 1) * BL, :]          # (2500, BL, I)
        xk = xk.transpose(2, 0, 1).reshape(I, XCOLS)
        xdiag = np.concatenate([xk, np.ones((1, XCOLS), np.float32)], 0)
        in_maps.append({
            "xdiag": np.ascontiguousarray(xdiag).astype(MM_NP),
            "wih": wih, "wth": wth, "wsh": wsh, "wc": wc, "bc": bc,
        })
    return in_maps


def run_on_device(in_maps):
    nc = _get_nc()
    res = run_bass_kernel_spmd(nc, in_maps, core_ids=list(range(NCORES)))
    return np.concatenate([res.results[k]["out"] for k in range(NCORES)], 0)


def kernel(data, W_ih, W_th, W_sh, b, weight_c, bias_c, batch_size=None,
           **_ignored):
    in_maps = _pack_inputs(data, W_ih, W_th, W_sh, b, weight_c, bias_c)
    return run_on_device(in_maps)


if __name__ == "__main__":
    d = np.load(os.path.join(os.path.dirname(__file__), "inputs.npz"))
    out = kernel(d["data"], d["W_ih"], d["W_th"], d["W_sh"], d["b"],
                 d["weight_c"], d["bias_c"])
    exp = np.load(os.path.join(os.path.dirname(__file__), "oracle64.npy"))
    aerr = np.abs(out - exp).max()
    print("absmax err vs fp64 oracle:", aerr,
          " rel:", aerr / np.abs(exp).max())


# revision 3
# speedup vs baseline: 26.3146x; 26.3146x over previous
"""Trainium2 Bass kernel for nn_GCAModel (2D ST-LSTM recurrence + classifier).

Strategy (per the batch-data-parallel hint + anti-diagonal wavefront):
  - Shard batch B=128 across 8 cores (16 rows each); weights replicated.
  - Within a core, process the (t, j) grid along anti-diagonals d = t + j.
    All cells on a diagonal are independent -> one fused GEMM per diagonal
    with moving dim N = n_cells * 16.
  - State h/c lives in SBUF as (128 part = H-chunk, 2 chunks, slot*16 cols),
    slot s holds row t = s-1 (slot 0 is a permanent zero guard). The GEMM for
    diagonal d reads the h_t operand at slot offset t_lo and the h_s operand
    at t_lo+1 -- the same buffer, shifted by one slot.
  - Gates computed as g.T: out (gate-chunk 128, N) = W.T chunk @ h-chunk,
    accumulating 5 K-groups in PSUM (x+bias K=4, Wth 2x128, Wsh 2x128).
  - ScalarE applies sigmoid/tanh (PSUM->SBUF), VectorE does the cell update,
    writing h/c back into the state buffer in place.
  - Mean-pool h via a wide fp32 accumulator += on each diagonal; classifier
    (+log_softmax) on device; output (16, 60) per core, concatenated on host.

Numerics: fp16 storage for x/weights/h/gates (PSUM accumulates fp32), fp32
cell state c (c grows beyond fp16 range; fp16-with-fp32-c measured 2.4e-4
rel err vs fp64 oracle on CPU).
"""
import os
os.environ.setdefault("JAX_PLATFORMS", "axon,cpu")

import numpy as np

import concourse.bass as bass
import concourse.tile as tile
from concourse import bacc, mybir
from concourse.bass_utils import run_bass_kernel_spmd

# ---------------------------------------------------------------- problem dims
T, J, B, I, H, C = 100, 25, 128, 3, 256, 60
NCORES = 8
BL = B // NCORES            # 16 batch rows per core
G5 = 5 * H                  # 1280 gate columns
SLOTS = T + 1               # +1 zero-guard slot at the front
SW = SLOTS * BL             # state width (free dim) per H-chunk
NMAX = min(T, J) * BL       # widest diagonal: 25*16 = 400

# gate order: process u first, o last (c needs i,fs,ft,u; h needs o + tanh(c))
GATES = [("u", 4 * H, "Tanh"), ("i", 0, "Sigmoid"), ("fs", H, "Sigmoid"),
         ("ft", 2 * H, "Sigmoid"), ("o", 3 * H, "Sigmoid")]

# diagonals: d = t + j
DIAGS = []
_off = 0
for _d in range(T + J - 1):
    _tlo, _thi = max(0, _d - (J - 1)), min(_d, T - 1)
    _nd = _thi - _tlo + 1
    DIAGS.append((_tlo, _nd, _off))
    _off += _nd * BL
XCOLS = _off                # 40000

# ---------------------------------------------------------------- dtype knobs
MM_DT = mybir.dt.float16      # x / W / h storage (matmul operands)
GATE_DT = mybir.dt.float16    # post-activation gates, t1, tanh(c)
C_DT = mybir.dt.float32       # cell state + c-proportional temps
MM_NP = np.float16

F32 = mybir.dt.float32


def _build_nc():
    nc = bacc.Bacc("TRN2", target_bir_lowering=False, debug=False,
                   num_devices=NCORES)
    x_d = nc.dram_tensor("xdiag", [4, XCOLS], MM_DT, kind="ExternalInput")
    wih_d = nc.dram_tensor("wih", [4, G5], MM_DT, kind="ExternalInput")
    wth_d = nc.dram_tensor("wth", [128, 2, G5], MM_DT, kind="ExternalInput")
    wsh_d = nc.dram_tensor("wsh", [128, 2, G5], MM_DT, kind="ExternalInput")
    wc_d = nc.dram_tensor("wc", [128, 2, C], F32, kind="ExternalInput")
    bc_d = nc.dram_tensor("bc", [1, C], F32, kind="ExternalInput")
    out_d = nc.dram_tensor("out", [BL, C], F32, kind="ExternalOutput")

    AF = mybir.ActivationFunctionType

    with tile.TileContext(nc) as tc:
        with tc.tile_pool(name="const", bufs=1) as const, \
             tc.tile_pool(name="state", bufs=1) as state, \
             tc.tile_pool(name="xin", bufs=6) as xin, \
             tc.tile_pool(name="gate", bufs=3) as gatep, \
             tc.tile_pool(name="work", bufs=3) as work, \
             tc.tile_pool(name="psg", bufs=3, space="PSUM") as psg, \
             tc.tile_pool(name="pscls", bufs=1, space="PSUM") as pscls:

            # ---- load constants
            wih_s = const.tile([4, G5], MM_DT)
            nc.sync.dma_start(out=wih_s, in_=wih_d[:, :])
            wth_s = const.tile([128, 2, G5], MM_DT)
            nc.sync.dma_start(out=wth_s, in_=wth_d[:, :, :])
            wsh_s = const.tile([128, 2, G5], MM_DT)
            nc.sync.dma_start(out=wsh_s, in_=wsh_d[:, :, :])
            wc_s = const.tile([128, 2, C], F32)
            nc.sync.dma_start(out=wc_s, in_=wc_d[:, :, :])
            bc_s = const.tile([1, C], F32)
            nc.sync.dma_start(out=bc_s, in_=bc_d[:, :])
            ones_s = const.tile([1, BL], F32)
            nc.vector.memset(ones_s, 1.0)

            # ---- state (slot 0 stays zero forever)
            h_st = state.tile([128, 2, SW], MM_DT)
            c_st = state.tile([128, 2, SW], C_DT)
            hsum = state.tile([128, 2, SW], F32)
            nc.vector.memset(h_st, 0.0)
            nc.gpsimd.memset(c_st, 0.0)
            nc.gpsimd.memset(hsum, 0.0)

            # ---- the wavefront
            for tlo, nd, xoff in DIAGS:
                N = nd * BL
                ht, hs = tlo * BL, (tlo + 1) * BL   # slot offsets (cols)

                xs = xin.tile([4, NMAX], MM_DT, tag="x")
                nc.sync.dma_start(out=xs[:, 0:N], in_=x_d[:, xoff:xoff + N])

                gt = {}
                for gname, gc, fn in GATES:
                    ps = psg.tile([128, 2, 512], F32, tag="ps")
                    for m in (0, 1):
                        mc = gc + m * 128
                        o = ps[:, m, 0:N]
                        nc.tensor.matmul(o, wih_s[:, mc:mc + 128], xs[:, 0:N],
                                         start=True, stop=False)
                        nc.tensor.matmul(o, wth_s[:, 0, mc:mc + 128],
                                         h_st[:, 0, ht:ht + N],
                                         start=False, stop=False)
                        nc.tensor.matmul(o, wth_s[:, 1, mc:mc + 128],
                                         h_st[:, 1, ht:ht + N],
                                         start=False, stop=False)
                        nc.tensor.matmul(o, wsh_s[:, 0, mc:mc + 128],
                                         h_st[:, 0, hs:hs + N],
                                         start=False, stop=False)
                        nc.tensor.matmul(o, wsh_s[:, 1, mc:mc + 128],
                                         h_st[:, 1, hs:hs + N],
                                         start=False, stop=True)
                    g = gatep.tile([128, 2, NMAX], GATE_DT, tag=gname)
                    nc.scalar.activation(out=g[:, :, 0:N], in_=ps[:, :, 0:N],
                                         func=getattr(AF, fn))
                    gt[gname] = g

                t1 = work.tile([128, 2, NMAX], GATE_DT, tag="t1")
                nc.vector.tensor_mul(t1[:, :, 0:N], gt["i"][:, :, 0:N],
                                     gt["u"][:, :, 0:N])
                t2 = work.tile([128, 2, NMAX], C_DT, tag="t2")
                nc.vector.tensor_mul(t2[:, :, 0:N], gt["fs"][:, :, 0:N],
                                     c_st[:, :, hs:hs + N])
                s12 = work.tile([128, 2, NMAX], C_DT, tag="s12")
                nc.vector.tensor_add(s12[:, :, 0:N], t1[:, :, 0:N],
                                     t2[:, :, 0:N])
                t3 = work.tile([128, 2, NMAX], C_DT, tag="t3")
                nc.vector.tensor_mul(t3[:, :, 0:N], gt["ft"][:, :, 0:N],
                                     c_st[:, :, ht:ht + N])
                nc.vector.tensor_add(c_st[:, :, hs:hs + N], s12[:, :, 0:N],
                                     t3[:, :, 0:N])
                tcz = work.tile([128, 2, NMAX], GATE_DT, tag="tc")
                nc.scalar.activation(out=tcz[:, :, 0:N],
                                     in_=c_st[:, :, hs:hs + N], func=AF.Tanh)
                nc.vector.tensor_mul(h_st[:, :, hs:hs + N],
                                     gt["o"][:, :, 0:N], tcz[:, :, 0:N])
                nc.vector.tensor_add(hsum[:, :, hs:hs + N],
                                     hsum[:, :, hs:hs + N],
                                     h_st[:, :, hs:hs + N])

            # ---- mean-pool: fold slots 1..100 down onto slot 1
            cur = T
            while cur > 1:
                if cur % 2 == 1:
                    last = BL + (cur - 1) * BL
                    nc.vector.tensor_add(hsum[:, :, BL:2 * BL],
                                         hsum[:, :, BL:2 * BL],
                                         hsum[:, :, last:last + BL])
                    cur -= 1
                half = cur // 2
                w = half * BL
                nc.vector.tensor_add(hsum[:, :, BL:BL + w],
                                     hsum[:, :, BL:BL + w],
                                     hsum[:, :, BL + w:BL + 2 * w])
                cur = half
            # F = hsum[:, :, BL:2*BL]  (128, 2, 16) fp32; 1/(T*J) folded into wc

            # ---- classifier logits.T? no: out (BL, C) with batch on partitions
            pc = pscls.tile([BL, 512], F32, tag="cls")
            lg = pc[:, 0:C]
            nc.tensor.matmul(lg, hsum[:, 0, BL:2 * BL], wc_s[:, 0, :],
                             start=True, stop=False)
            nc.tensor.matmul(lg, hsum[:, 1, BL:2 * BL], wc_s[:, 1, :],
                             start=False, stop=False)
            nc.tensor.matmul(lg, ones_s[:, :], bc_s[:, :],
                             start=False, stop=True)

            # ---- log_softmax over free dim
            mx = work.tile([BL, 1], F32, tag="mx")
            nc.vector.reduce_max(out=mx, in_=lg, axis=mybir.AxisListType.X)
            nmx = work.tile([BL, 1], F32, tag="nmx")
            nc.scalar.mul(out=nmx, in_=mx, mul=-1.0)
            ex = work.tile([BL, C], F32, tag="ex")
            nc.scalar.activation(out=ex, in_=lg, func=AF.Exp, bias=nmx)
            sm = work.tile([BL, 1], F32, tag="sm")
            nc.vector.reduce_sum(out=sm, in_=ex, axis=mybir.AxisListType.X)
            lse = work.tile([BL, 1], F32, tag="lse")
            nc.scalar.activation(out=lse, in_=sm, func=AF.Ln)
            tot = work.tile([BL, 1], F32, tag="tot")
            nc.vector.tensor_add(tot, mx, lse)
            res = work.tile([BL, C], F32, tag="res")
            nc.vector.tensor_scalar(out=res, in0=lg, scalar1=tot, scalar2=None,
                                    op0=mybir.AluOpType.subtract)
            nc.sync.dma_start(out=out_d[:, :], in_=res)

    nc.compile()
    return nc


_NC = None


def _get_nc():
    global _NC
    if _NC is None:
        _NC = _build_nc()
    return _NC


def _pack_inputs(data, W_ih, W_th, W_sh, b, weight_c, bias_c):
    """Host-side prep: weights in lhsT layout, x in diagonal-major order."""
    data = np.asarray(data, np.float32)
    # lhsT for the x-GEMM: (I+1, 1280) = [W_ih.T; b] (bias via ones row in x)
    wih = np.concatenate([np.asarray(W_ih, np.float32).T,
                          np.asarray(b, np.float32)[None, :]], 0).astype(MM_NP)
    # lhsT for h-GEMMs: (128, chunk, 1280)
    wth = np.asarray(W_th, np.float32).T.reshape(2, 128, G5).transpose(1, 0, 2)
    wsh = np.asarray(W_sh, np.float32).T.reshape(2, 128, G5).transpose(1, 0, 2)
    wth = np.ascontiguousarray(wth).astype(MM_NP)
    wsh = np.ascontiguousarray(wsh).astype(MM_NP)
    # classifier: fold the 1/(T*J) mean into the weights
    wc = (np.asarray(weight_c, np.float32).T / (T * J)).reshape(2, 128, C)
    wc = np.ascontiguousarray(wc.transpose(1, 0, 2), np.float32)
    bc = np.asarray(bias_c, np.float32)[None, :]

    # x in diagonal-major order: cols (cell-in-diag, batch), rows (I..., ones)
    tt = np.concatenate([np.arange(max(0, d - (J - 1)), min(d, T - 1) + 1)
                         for d in range(T + J - 1)])
    jj = np.concatenate([d - np.arange(max(0, d - (J - 1)), min(d, T - 1) + 1)
                         for d in range(T + J - 1)])
    xc = data[tt, jj]                     # (2500, B, I)
    in_maps = []
    for k in range(NCORES):
        xk = xc[:, k * BL:(k + 1) * BL, :]          # (2500, BL, I)
        xk = xk.transpose(2, 0, 1).reshape(I, XCOLS)
        xdiag = np.concatenate([xk, np.ones((1, XCOLS), np.float32)], 0)
        in_maps.append({
            "xdiag": np.ascontiguousarray(xdiag).astype(MM_NP),
            "wih": wih, "wth": wth, "wsh": wsh, "wc": wc, "bc": bc,
        })
    return in_maps


class _Runner:
    """Persistent jitted SPMD executable (run_bass_via_pjrt traces+jits on
    every call; this caches the jit and keeps inputs device-resident)."""

    def __init__(self, nc):
        import jax
        from jax.sharding import Mesh, PartitionSpec
        from jax.experimental.shard_map import shard_map
        from concourse import mybir as _mb
        from concourse.bass2jax import _bass_exec_p, install_neuronx_cc_hook

        install_neuronx_cc_hook()
        in_names, out_names, out_avals, zero_outs = [], [], [], []
        for alloc in nc.m.functions[0].allocations:
            if not isinstance(alloc, _mb.MemoryLocationSet):
                continue
            name = alloc.memorylocations[0].name
            if alloc.kind == "ExternalInput":
                in_names.append(name)
            elif alloc.kind == "ExternalOutput":
                out_names.append(name)
                shape = tuple(alloc.tensor_shape)
                dtype = _mb.dt.np(alloc.dtype)
                out_avals.append(jax.core.ShapedArray(shape, dtype))
                zero_outs.append(np.zeros(shape, dtype))
        self.in_names, self.out_names = in_names, out_names
        n_params, n_outs = len(in_names), len(out_names)
        all_names = tuple(in_names + out_names)

        def _body(*args):
            return tuple(_bass_exec_p.bind(
                *args, out_avals=tuple(out_avals), in_names=all_names,
                out_names=tuple(out_names), lowering_input_output_aliases=(),
                sim_require_finite=True, sim_require_nnan=True, nc=nc))

        devices = jax.devices()[:NCORES]
        self.mesh = Mesh(np.asarray(devices), ("core",))
        in_specs = (PartitionSpec("core"),) * (n_params + n_outs)
        out_specs = (PartitionSpec("core"),) * n_outs
        self._jit = jax.jit(
            shard_map(_body, mesh=self.mesh, in_specs=in_specs,
                      out_specs=out_specs, check_rep=False),
            donate_argnums=tuple(range(n_params, n_params + n_outs)),
            keep_unused=True)
        self._zeros = zero_outs
        self._dev_in = None
        self._jax = jax

    def put_inputs(self, in_maps):
        import jax
        from jax.sharding import NamedSharding, PartitionSpec
        sh = NamedSharding(self.mesh, PartitionSpec("core"))
        nc = _get_nc()
        pid = nc.partition_id_tensor.name if nc.partition_id_tensor else None
        in_maps = [dict(m) for m in in_maps]
        for k, m in enumerate(in_maps):
            if pid is not None:
                m[pid] = np.array([[k]], dtype=np.uint32)
        self._dev_in = [
            jax.device_put(np.concatenate(
                [np.asarray(m[n]) for m in in_maps], 0), sh)
            for n in self.in_names]

    def run(self):
        zeros = [np.concatenate([z] * NCORES, 0) for z in self._zeros]
        outs = self._jit(*self._dev_in, *zeros)
        return [np.asarray(o) for o in outs]


_RUNNER = None


def _get_runner():
    global _RUNNER
    if _RUNNER is None:
        _RUNNER = _Runner(_get_nc())
    return _RUNNER


def run_on_device(in_maps):
    r = _get_runner()
    r.put_inputs(in_maps)
    out = r.run()[0]          # (8*BL, C) concat over cores
    return out.reshape(NCORES * BL, C)


def kernel(data, W_ih, W_th, W_sh, b, weight_c, bias_c, batch_size=None,
           **_ignored):
    in_maps = _pack_inputs(data, W_ih, W_th, W_sh, b, weight_c, bias_c)
    return run_on_device(in_maps)


if __name__ == "__main__":
    d = np.load(os.path.join(os.path.dirname(__file__), "inputs.npz"))
    out = kernel(d["data"], d["W_ih"], d["W_th"], d["W_sh"], d["b"],
                 d["weight_c"], d["bias_c"])
    exp = np.load(os.path.join(os.path.dirname(__file__), "oracle64.npy"))
    aerr = np.abs(out - exp).max()
    print("absmax err vs fp64 oracle:", aerr,
          " rel:", aerr / np.abs(exp).max())


# revision 24
# speedup vs baseline: 1044.8414x; 39.7057x over previous
"""Trainium2 Bass kernel for nn_GCAModel (2D ST-LSTM recurrence + classifier).

Strategy (per the batch-data-parallel hint + anti-diagonal wavefront):
  - Shard batch B=128 across 8 cores (16 rows each); weights replicated.
  - Within a core, process the (t, j) grid along anti-diagonals d = t + j.
    All cells on a diagonal are independent -> one fused GEMM per diagonal
    with moving dim N = n_cells * 16.
  - Each diagonal is further split into two sub-waves (H = the 13 cells on
    the growing edge, L = the rest) chosen so that H(d+1) depends only on
    H(d) and L(d+1) only on {L(d), H(d)}. Processing order ...H(d) L(d)
    H(d+1) L(d+1)... then lets the ACT/DVE tail of one sub-wave hide under
    the PE GEMMs of the next -> near-full tensor-engine occupancy despite
    the serial recurrence.
  - State h/c lives in SBUF as (128 part = H-chunk, 2 chunks, slot*16 cols),
    slot s holds row t = s-1 (slot 0 is a permanent zero guard). The GEMM for
    a (sub-)wave starting at row t0 reads the h_t operand at slot offset t0
    and the h_s operand at t0+1 -- the same buffer shifted by one slot.
  - Gates computed as g.T: out (gate-chunk 128, N) = W.T chunk @ h-chunk,
    accumulating 5 K-groups in one PSUM bank per gate (x+bias K=4, Wth
    2x128, Wsh 2x128).
  - ScalarE applies sigmoid/tanh (PSUM->SBUF), VectorE does the cell update
    in place; mean-pool h via a wide fp32 accumulator once per diagonal;
    classifier + log_softmax on device; (16, 60) per core, host concat.

Numerics: fp16 storage for x/weights/h/gates (PSUM accumulates fp32), fp32
cell state c (c grows beyond fp16 range; fp16-with-fp32-c measured 2.4e-4
rel err vs fp64 oracle on CPU).
"""
import os
os.environ.setdefault("JAX_PLATFORMS", "axon,cpu")

import numpy as np

import concourse.bass as bass
import concourse.tile as tile
from concourse import bacc, mybir
from concourse.bass_utils import run_bass_kernel_spmd

# ---------------------------------------------------------------- problem dims
T, J, B, I, H, C = 100, 25, 128, 3, 256, 60
NCORES = 8
BL = B // NCORES            # 16 batch rows per core
G5 = 5 * H                  # 1280 gate columns
SLOTS = T + 1               # +1 zero-guard slot at the front
SW = SLOTS * BL             # state width (free dim) per H-chunk
NMAX = min(T, J) * BL       # widest diagonal: 25*16 = 400
SUBMAX = 13                 # max sub-wave cells
NSUB = SUBMAX * BL          # 208 cols

# gate order: u first, o last (c needs i,fs,ft,u; h needs o + tanh(c))
GATES = [("u", 4 * H, "Tanh"), ("i", 0, "Sigmoid"), ("fs", H, "Sigmoid"),
         ("ft", 2 * H, "Sigmoid"), ("o", 3 * H, "Sigmoid")]

# diagonals: d = t + j; per diagonal the active rows are [tlo, thi]
DIAGS = []
_off = 0
for _d in range(T + J - 1):
    _tlo, _thi = max(0, _d - (J - 1)), min(_d, T - 1)
    _nd = _thi - _tlo + 1
    DIAGS.append((_d, _tlo, _nd, _off))
    _off += _nd * BL
XCOLS = _off                # 40000


def _subwaves(d, tlo, nd):
    """Sub-waves in processing order (top/growing-edge first).

    Split so that H(d+1) reads only slots written by H(d) (+ zero-init),
    and L(d+1) only slots from L(d) and H(d). Verified by construction:
    grow phase anchors H to the top 13 rows, shrink phase anchors L to the
    bottom 13 rows.
    """
    thi = tlo + nd - 1
    if nd <= SUBMAX:
        return [(tlo, nd)]
    m = (thi - SUBMAX) if d <= 99 else (tlo + SUBMAX - 1)
    return [(m + 1, thi - m), (tlo, m - tlo + 1)]


# ---------------------------------------------------------------- dtype knobs
MM_DT = mybir.dt.float16      # x / W / h storage (matmul operands)
GATE_DT = mybir.dt.float16    # post-activation gates, t1, tanh(c)
C_DT = mybir.dt.bfloat16      # cell state + c-temps (c exceeds fp16 range;
                              # fp16 range; bf16 keeps DVE in 2x mode and
                              # costs 2.6e-3 absmax vs fp32-c's 0.9e-3)
MM_NP = np.float16
# row-group packing of the x-GEMMs: 0=off, 1=all, 2=first-chunk only
XPACK = int(os.environ.get("KERNEL_XPACK", "0"))

F32 = mybir.dt.float32


def _build_nc(reps=1):
    nc = bacc.Bacc("TRN2", target_bir_lowering=False, debug=False,
                   num_devices=NCORES)
    # x + wih carry 4 replicas (rows 4r..4r+3) so the K=4 x-GEMMs can run on
    # four disjoint 32-row PE sub-arrays concurrently (tile_position packing)
    x_d = nc.dram_tensor("xdiag", [16, XCOLS], MM_DT, kind="ExternalInput")
    wih_d = nc.dram_tensor("wih", [16, G5], MM_DT, kind="ExternalInput")
    wth_d = nc.dram_tensor("wth", [128, 2, G5], MM_DT, kind="ExternalInput")
    wsh_d = nc.dram_tensor("wsh", [128, 2, G5], MM_DT, kind="ExternalInput")
    wc_d = nc.dram_tensor("wc", [128, 2, C], F32, kind="ExternalInput")
    bc_d = nc.dram_tensor("bc", [1, C], F32, kind="ExternalInput")
    out_d = nc.dram_tensor("out", [BL, C], F32, kind="ExternalOutput")

    AF = mybir.ActivationFunctionType

    with tile.TileContext(nc) as tc:
        with tc.tile_pool(name="const", bufs=1) as const, \
             tc.tile_pool(name="state", bufs=1) as state, \
             tc.tile_pool(name="gate", bufs=4) as gatep, \
             tc.tile_pool(name="work", bufs=4) as work, \
             tc.tile_pool(name="psg", bufs=7, space="PSUM") as psg, \
             tc.tile_pool(name="pscls", bufs=1, space="PSUM") as pscls:

            # ---- load constants (x and wih replicated at partitions 32r)
            wih_s = const.tile([100, G5], MM_DT)
            xs_all = const.tile([100, XCOLS], MM_DT)
            for r in range(4):
                nc.sync.dma_start(out=wih_s[32 * r:32 * r + 4, :],
                                  in_=wih_d[4 * r:4 * r + 4, :])
                nc.sync.dma_start(out=xs_all[32 * r:32 * r + 4, :],
                                  in_=x_d[4 * r:4 * r + 4, :])
            wth_s = const.tile([128, 2, G5], MM_DT)
            nc.sync.dma_start(out=wth_s, in_=wth_d[:, :, :])
            wsh_s = const.tile([128, 2, G5], MM_DT)
            nc.sync.dma_start(out=wsh_s, in_=wsh_d[:, :, :])
            wc_s = const.tile([128, 2, C], F32)
            nc.sync.dma_start(out=wc_s, in_=wc_d[:, :, :])
            bc_s = const.tile([1, C], F32)
            nc.sync.dma_start(out=bc_s, in_=bc_d[:, :])
            ones_s = const.tile([1, BL], F32)
            nc.vector.memset(ones_s, 1.0)

            # ---- state (slot 0 stays zero forever)
            h_st = state.tile([128, 2, SW], MM_DT)
            c_st = state.tile([128, 2, SW], C_DT)
            hsum = state.tile([128, 2, SW], F32)

            # (reps>1 replicates the computation for slope-timing)
            for _rep in range(reps):
                nc.vector.memset(h_st, 0.0)
                nc.gpsimd.memset(c_st, 0.0)
                nc.gpsimd.memset(hsum, 0.0)

                for d, tlo, nd, xoff in DIAGS:
                    N = nd * BL

                    for stlo, snd in _subwaves(d, tlo, nd):
                        Nw = snd * BL
                        ht, hs = stlo * BL, (stlo + 1) * BL
                        xc = xoff + (stlo - tlo) * BL

                        # x-GEMMs first, row-group packed 4-wide: distinct
                        # PSUM banks + distinct 32-row sub-arrays -> the PE
                        # runs them concurrently (and ahead, during the
                        # previous sub-wave's ACT/DVE tail)
                        ps = {}
                        for gname, _, _ in GATES:
                            ps[gname] = psg.tile([128, 2, 256], F32,
                                                 name="ps_" + gname, tag="ps")
                        # this order gives every consecutive 4 jobs distinct
                        # PSUM banks and (via idx%4) distinct row-groups
                        xjobs = [(g, m) for m in (0, 1) for g, _, _ in GATES]
                        gcol = dict((g, gc) for g, gc, _ in GATES)
                        for idx, (gname, m) in enumerate(xjobs):
                            packed = XPACK >= 1 and (m == 0 or XPACK == 1)
                            r = idx % 4 if packed else 0
                            mc = gcol[gname] + m * 128
                            # start=True clears has_written for the WHOLE
                            # bank -> only the first matmul touching each
                            # gate's (single-bank) tile may carry it
                            nc.tensor.matmul(
                                ps[gname][:, m, 0:Nw],
                                wih_s[32 * r:32 * r + 4, mc:mc + 128],
                                xs_all[32 * r:32 * r + 4, xc:xc + Nw],
                                start=(m == 0), stop=False,
                                tile_position=(32 * r, 0) if packed else None)

                        gt = {}
                        for gname, gc, fn in GATES:
                            for m in (0, 1):
                                mc = gc + m * 128
                                o = ps[gname][:, m, 0:Nw]
                                nc.tensor.matmul(
                                    o, wth_s[:, 0, mc:mc + 128],
                                    h_st[:, 0, ht:ht + Nw],
                                    start=False, stop=False)
                                nc.tensor.matmul(
                                    o, wth_s[:, 1, mc:mc + 128],
                                    h_st[:, 1, ht:ht + Nw],
                                    start=False, stop=False)
                                nc.tensor.matmul(
                                    o, wsh_s[:, 0, mc:mc + 128],
                                    h_st[:, 0, hs:hs + Nw],
                                    start=False, stop=False)
                                nc.tensor.matmul(
                                    o, wsh_s[:, 1, mc:mc + 128],
                                    h_st[:, 1, hs:hs + Nw],
                                    start=False, stop=True)
                            g = gatep.tile([128, 2, NSUB], GATE_DT, tag=gname)
                            nc.scalar.activation(out=g[:, :, 0:Nw],
                                                 in_=ps[gname][:, :, 0:Nw],
                                                 func=getattr(AF, fn))
                            gt[gname] = g

                        t1 = work.tile([128, 2, NSUB], GATE_DT, tag="t1")
                        nc.vector.tensor_mul(t1[:, :, 0:Nw],
                                             gt["i"][:, :, 0:Nw],
                                             gt["u"][:, :, 0:Nw])
                        t2 = work.tile([128, 2, NSUB], C_DT, tag="t2")
                        nc.vector.tensor_mul(t2[:, :, 0:Nw],
                                             gt["fs"][:, :, 0:Nw],
                                             c_st[:, :, hs:hs + Nw])
                        s12 = work.tile([128, 2, NSUB], C_DT, tag="s12")
                        nc.vector.tensor_add(s12[:, :, 0:Nw], t1[:, :, 0:Nw],
                                             t2[:, :, 0:Nw])
                        t3 = work.tile([128, 2, NSUB], C_DT, tag="t3")
                        nc.vector.tensor_mul(t3[:, :, 0:Nw],
                                             gt["ft"][:, :, 0:Nw],
                                             c_st[:, :, ht:ht + Nw])
                        nc.vector.tensor_add(c_st[:, :, hs:hs + Nw],
                                             s12[:, :, 0:Nw], t3[:, :, 0:Nw])
                        tcz = work.tile([128, 2, NSUB], GATE_DT, tag="tc")
                        nc.scalar.activation(out=tcz[:, :, 0:Nw],
                                             in_=c_st[:, :, hs:hs + Nw],
                                             func=AF.Tanh)
                        nc.vector.tensor_mul(h_st[:, :, hs:hs + Nw],
                                             gt["o"][:, :, 0:Nw],
                                             tcz[:, :, 0:Nw])

                    # one mean-pool accumulate for the whole diagonal
                    lo = (tlo + 1) * BL
                    nc.vector.tensor_add(hsum[:, :, lo:lo + N],
                                         hsum[:, :, lo:lo + N],
                                         h_st[:, :, lo:lo + N])

                # ---- mean-pool: fold slots 1..100 down onto slot 1
                cur = T
                while cur > 1:
                    if cur % 2 == 1:
                        last = BL + (cur - 1) * BL
                        nc.vector.tensor_add(hsum[:, :, BL:2 * BL],
                                             hsum[:, :, BL:2 * BL],
                                             hsum[:, :, last:last + BL])
                        cur -= 1
                    half = cur // 2
                    w = half * BL
                    nc.vector.tensor_add(hsum[:, :, BL:BL + w],
                                         hsum[:, :, BL:BL + w],
                                         hsum[:, :, BL + w:BL + 2 * w])
                    cur = half
                # F = hsum[:, :, BL:2*BL] (128, 2, 16) fp32; 1/(T*J) in wc

                # ---- classifier: out (BL, C), batch on partitions
                pc = pscls.tile([BL, 512], F32, tag="cls")
                lg = pc[:, 0:C]
                nc.tensor.matmul(lg, hsum[:, 0, BL:2 * BL], wc_s[:, 0, :],
                                 start=True, stop=False)
                nc.tensor.matmul(lg, hsum[:, 1, BL:2 * BL], wc_s[:, 1, :],
                                 start=False, stop=False)
                nc.tensor.matmul(lg, ones_s[:, :], bc_s[:, :],
                                 start=False, stop=True)

                # ---- log_softmax over free dim
                mx = work.tile([BL, 1], F32, tag="mx")
                nc.vector.reduce_max(out=mx, in_=lg,
                                     axis=mybir.AxisListType.X)
                nmx = work.tile([BL, 1], F32, tag="nmx")
                nc.scalar.mul(out=nmx, in_=mx, mul=-1.0)
                ex = work.tile([BL, C], F32, tag="ex")
                nc.scalar.activation(out=ex, in_=lg, func=AF.Exp, bias=nmx)
                sm = work.tile([BL, 1], F32, tag="sm")
                nc.vector.reduce_sum(out=sm, in_=ex,
                                     axis=mybir.AxisListType.X)
                lse = work.tile([BL, 1], F32, tag="lse")
                nc.scalar.activation(out=lse, in_=sm, func=AF.Ln)
                tot = work.tile([BL, 1], F32, tag="tot")
                nc.vector.tensor_add(tot, mx, lse)
                res = work.tile([BL, C], F32, tag="res")
                nc.vector.tensor_scalar(out=res, in0=lg, scalar1=tot,
                                        scalar2=None,
                                        op0=mybir.AluOpType.subtract)
                nc.sync.dma_start(out=out_d[:, :], in_=res)

    nc.compile()
    return nc


_NC = None


def _get_nc():
    global _NC
    if _NC is None:
        _NC = _build_nc()
    return _NC


def _pack_inputs(data, W_ih, W_th, W_sh, b, weight_c, bias_c):
    """Host-side prep: weights in lhsT layout, x in diagonal-major order."""
    data = np.asarray(data, np.float32)
    # lhsT for the x-GEMM: (I+1, 1280) = [W_ih.T; b] (bias via ones row in x),
    # replicated 4x for row-group packing
    wih = np.concatenate([np.asarray(W_ih, np.float32).T,
                          np.asarray(b, np.float32)[None, :]], 0).astype(MM_NP)
    wih = np.tile(wih, (4, 1))
    # lhsT for h-GEMMs: (128, chunk, 1280)
    wth = np.asarray(W_th, np.float32).T.reshape(2, 128, G5).transpose(1, 0, 2)
    wsh = np.asarray(W_sh, np.float32).T.reshape(2, 128, G5).transpose(1, 0, 2)
    wth = np.ascontiguousarray(wth).astype(MM_NP)
    wsh = np.ascontiguousarray(wsh).astype(MM_NP)
    # classifier: fold the 1/(T*J) mean into the weights
    wc = (np.asarray(weight_c, np.float32).T / (T * J)).reshape(2, 128, C)
    wc = np.ascontiguousarray(wc.transpose(1, 0, 2), np.float32)
    bc = np.asarray(bias_c, np.float32)[None, :]

    # x in diagonal-major order: cols (cell-in-diag, batch), rows (I..., ones)
    tt = np.concatenate([np.arange(max(0, d - (J - 1)), min(d, T - 1) + 1)
                         for d in range(T + J - 1)])
    jj = np.concatenate([d - np.arange(max(0, d - (J - 1)), min(d, T - 1) + 1)
                         for d in range(T + J - 1)])
    xc = data[tt, jj]                     # (2500, B, I)
    in_maps = []
    for k in range(NCORES):
        xk = xc[:, k * BL:(k + 1) * BL, :]          # (2500, BL, I)
        xk = xk.transpose(2, 0, 1).reshape(I, XCOLS)
        xdiag = np.concatenate([xk, np.ones((1, XCOLS), np.float32)], 0)
        xdiag = np.tile(xdiag, (4, 1))
        in_maps.append({
            "xdiag": np.ascontiguousarray(xdiag).astype(MM_NP),
            "wih": wih, "wth": wth, "wsh": wsh, "wc": wc, "bc": bc,
        })
    return in_maps


class _Runner:
    """Persistent jitted SPMD executable (run_bass_via_pjrt traces+jits on
    every call; this caches the jit and keeps inputs device-resident)."""

    def __init__(self, nc):
        import jax
        from jax.sharding import Mesh, PartitionSpec
        from jax.experimental.shard_map import shard_map
        from concourse import mybir as _mb
        from concourse.bass2jax import _bass_exec_p, install_neuronx_cc_hook

        install_neuronx_cc_hook()
        self.nc = nc
        in_names, out_names, out_avals, zero_outs = [], [], [], []
        for alloc in nc.m.functions[0].allocations:
            if not isinstance(alloc, _mb.MemoryLocationSet):
                continue
            name = alloc.memorylocations[0].name
            if alloc.kind == "ExternalInput":
                in_names.append(name)
            elif alloc.kind == "ExternalOutput":
                out_names.append(name)
                shape = tuple(alloc.tensor_shape)
                dtype = _mb.dt.np(alloc.dtype)
                out_avals.append(jax.core.ShapedArray(shape, dtype))
                zero_outs.append(np.zeros(shape, dtype))
        self.in_names, self.out_names = in_names, out_names
        n_params, n_outs = len(in_names), len(out_names)
        all_names = tuple(in_names + out_names)

        def _body(*args):
            return tuple(_bass_exec_p.bind(
                *args, out_avals=tuple(out_avals), in_names=all_names,
                out_names=tuple(out_names), lowering_input_output_aliases=(),
                sim_require_finite=True, sim_require_nnan=True, nc=nc))

        devices = jax.devices()[:NCORES]
        self.mesh = Mesh(np.asarray(devices), ("core",))
        in_specs = (PartitionSpec("core"),) * (n_params + n_outs)
        out_specs = (PartitionSpec("core"),) * n_outs
        self._jit = jax.jit(
            shard_map(_body, mesh=self.mesh, in_specs=in_specs,
                      out_specs=out_specs, check_rep=False),
            donate_argnums=tuple(range(n_params, n_params + n_outs)),
            keep_unused=True)
        self._zeros = zero_outs
        self._dev_in = None

    def put_inputs(self, in_maps):
        import jax
        from jax.sharding import NamedSharding, PartitionSpec
        sh = NamedSharding(self.mesh, PartitionSpec("core"))
        pid = (self.nc.partition_id_tensor.name
               if self.nc.partition_id_tensor else None)
        in_maps = [dict(m) for m in in_maps]
        for k, m in enumerate(in_maps):
            if pid is not None:
                m[pid] = np.array([[k]], dtype=np.uint32)
        self._dev_in = [
            jax.device_put(np.concatenate(
                [np.asarray(m[n]) for m in in_maps], 0), sh)
            for n in self.in_names]

    def run(self):
        zeros = [np.concatenate([z] * NCORES, 0) for z in self._zeros]
        outs = self._jit(*self._dev_in, *zeros)
        return [np.asarray(o) for o in outs]


_RUNNER = None


def _get_runner():
    global _RUNNER
    if _RUNNER is None:
        _RUNNER = _Runner(_get_nc())
    return _RUNNER


def run_on_device(in_maps):
    r = _get_runner()
    r.put_inputs(in_maps)
    out = r.run()[0]          # (8*BL, C) concat over cores
    return out.reshape(NCORES * BL, C)


def kernel(data, W_ih, W_th, W_sh, b, weight_c, bias_c, batch_size=None,
           **_ignored):
    in_maps = _pack_inputs(data, W_ih, W_th, W_sh, b, weight_c, bias_c)
    return run_on_device(in_maps)


if __name__ == "__main__":
    d = np.load(os.path.join(os.path.dirname(__file__), "inputs.npz"))
    out = kernel(d["data"], d["W_ih"], d["W_th"], d["W_sh"], d["b"],
                 d["weight_c"], d["bias_c"])
    exp = np.load(os.path.join(os.path.dirname(__file__), "oracle64.npy"))
    aerr = np.abs(out - exp).max()
    print("absmax err vs fp64 oracle:", aerr,
          " rel:", aerr / np.abs(exp).max())


# revision 25
# speedup vs baseline: 1699.4820x; 1.6265x over previous
"""Trainium2 Bass kernel for nn_GCAModel (2D ST-LSTM recurrence + classifier).

Strategy (per the batch-data-parallel hint + anti-diagonal wavefront):
  - Shard batch B=128 across 8 cores (16 rows each); weights replicated.
  - Within a core, process the (t, j) grid along anti-diagonals d = t + j.
    All cells on a diagonal are independent -> one fused GEMM per diagonal
    with moving dim N = n_cells * 16.
  - Each diagonal is further split into two sub-waves (H = the 13 cells on
    the growing edge, L = the rest) chosen so that H(d+1) depends only on
    H(d) and L(d+1) only on {L(d), H(d)}. Processing order ...H(d) L(d)
    H(d+1) L(d+1)... then lets the ACT/DVE tail of one sub-wave hide under
    the PE GEMMs of the next -> near-full tensor-engine occupancy despite
    the serial recurrence.
  - State h/c lives in SBUF as (128 part = H-chunk, 2 chunks, slot*16 cols),
    slot s holds row t = s-1 (slot 0 is a permanent zero guard). The GEMM for
    a (sub-)wave starting at row t0 reads the h_t operand at slot offset t0
    and the h_s operand at t0+1 -- the same buffer shifted by one slot.
  - Gates computed as g.T: out (gate-chunk 128, N) = W.T chunk @ h-chunk,
    accumulating 5 K-groups in one PSUM bank per gate (x+bias K=4, Wth
    2x128, Wsh 2x128).
  - ScalarE applies sigmoid/tanh (PSUM->SBUF), VectorE does the cell update
    in place; mean-pool h via a wide fp32 accumulator once per diagonal;
    classifier + log_softmax on device; (16, 60) per core, host concat.

Numerics: fp16 storage for x/weights/h/gates (PSUM accumulates fp32), fp32
cell state c (c grows beyond fp16 range; fp16-with-fp32-c measured 2.4e-4
rel err vs fp64 oracle on CPU).
"""
import os
os.environ.setdefault("JAX_PLATFORMS", "axon,cpu")

import numpy as np

import concourse.bass as bass
import concourse.tile as tile
from concourse import bacc, mybir
from concourse.bass_utils import run_bass_kernel_spmd

# ---------------------------------------------------------------- problem dims
T, J, B, I, H, C = 100, 25, 128, 3, 256, 60
NCORES = 8
BL = B // NCORES            # 16 batch rows per core
G5 = 5 * H                  # 1280 gate columns
SLOTS = T + 1               # +1 zero-guard slot at the front
SW = SLOTS * BL             # state width (free dim) per H-chunk
NMAX = min(T, J) * BL       # widest diagonal: 25*16 = 400
SUBMAX = 13                 # max sub-wave cells
NSUB = SUBMAX * BL          # 208 cols

# gate order: u first, o last (c needs i,fs,ft,u; h needs o + tanh(c))
GATES = [("u", 4 * H, "Tanh"), ("i", 0, "Sigmoid"), ("fs", H, "Sigmoid"),
         ("ft", 2 * H, "Sigmoid"), ("o", 3 * H, "Sigmoid")]

# diagonals: d = t + j; per diagonal the active rows are [tlo, thi]
DIAGS = []
_off = 0
for _d in range(T + J - 1):
    _tlo, _thi = max(0, _d - (J - 1)), min(_d, T - 1)
    _nd = _thi - _tlo + 1
    DIAGS.append((_d, _tlo, _nd, _off))
    _off += _nd * BL
XCOLS = _off                # 40000


def _subwaves(d, tlo, nd):
    """Sub-waves in processing order (top/growing-edge first).

    Split so that H(d+1) reads only slots written by H(d) (+ zero-init),
    and L(d+1) only slots from L(d) and H(d). Verified by construction:
    grow phase anchors H to the top 13 rows, shrink phase anchors L to the
    bottom 13 rows.
    """
    thi = tlo + nd - 1
    if nd <= SUBMAX:
        return [(tlo, nd)]
    m = (thi - SUBMAX) if d <= 99 else (tlo + SUBMAX - 1)
    return [(m + 1, thi - m), (tlo, m - tlo + 1)]


# ---------------------------------------------------------------- dtype knobs
MM_DT = mybir.dt.float16      # x / W / h storage (matmul operands)
GATE_DT = mybir.dt.float16    # post-activation gates, t1, tanh(c)
C_DT = mybir.dt.bfloat16      # cell state + c-temps (c exceeds fp16 range;
                              # fp16 range; bf16 keeps DVE in 2x mode and
                              # costs 2.6e-3 absmax vs fp32-c's 0.9e-3)
MM_NP = np.float16
# row-group packing of the x-GEMMs: 0=off, 1=all, 2=first-chunk only
XPACK = int(os.environ.get("KERNEL_XPACK", "0"))

F32 = mybir.dt.float32


def _build_nc(reps=1):
    nc = bacc.Bacc("TRN2", target_bir_lowering=False, debug=False,
                   num_devices=NCORES)
    # x + wih carry 4 replicas (rows 4r..4r+3) so the K=4 x-GEMMs can run on
    # four disjoint 32-row PE sub-arrays concurrently (tile_position packing)
    x_d = nc.dram_tensor("xdiag", [16, XCOLS], MM_DT, kind="ExternalInput")
    wih_d = nc.dram_tensor("wih", [16, G5], MM_DT, kind="ExternalInput")
    wth_d = nc.dram_tensor("wth", [128, 2, G5], MM_DT, kind="ExternalInput")
    wsh_d = nc.dram_tensor("wsh", [128, 2, G5], MM_DT, kind="ExternalInput")
    wc_d = nc.dram_tensor("wc", [128, 2, C], F32, kind="ExternalInput")
    bc_d = nc.dram_tensor("bc", [1, C], F32, kind="ExternalInput")
    out_d = nc.dram_tensor("out", [BL, C], F32, kind="ExternalOutput")

    AF = mybir.ActivationFunctionType

    with tile.TileContext(nc) as tc:
        with tc.tile_pool(name="const", bufs=1) as const, \
             tc.tile_pool(name="state", bufs=1) as state, \
             tc.tile_pool(name="gate", bufs=4) as gatep, \
             tc.tile_pool(name="work", bufs=4) as work, \
             tc.tile_pool(name="psg", bufs=7, space="PSUM") as psg, \
             tc.tile_pool(name="pscls", bufs=1, space="PSUM") as pscls:

            # ---- load constants (x and wih replicated at partitions 32r)
            wih_s = const.tile([100, G5], MM_DT)
            xs_all = const.tile([100, XCOLS], MM_DT)
            for r in range(4):
                nc.sync.dma_start(out=wih_s[32 * r:32 * r + 4, :],
                                  in_=wih_d[4 * r:4 * r + 4, :])
                nc.sync.dma_start(out=xs_all[32 * r:32 * r + 4, :],
                                  in_=x_d[4 * r:4 * r + 4, :])
            wth_s = const.tile([128, 2, G5], MM_DT)
            nc.sync.dma_start(out=wth_s, in_=wth_d[:, :, :])
            wsh_s = const.tile([128, 2, G5], MM_DT)
            nc.sync.dma_start(out=wsh_s, in_=wsh_d[:, :, :])
            wc_s = const.tile([128, 2, C], F32)
            nc.sync.dma_start(out=wc_s, in_=wc_d[:, :, :])
            bc_s = const.tile([1, C], F32)
            nc.sync.dma_start(out=bc_s, in_=bc_d[:, :])
            ones_s = const.tile([1, BL], F32)
            nc.vector.memset(ones_s, 1.0)

            # ---- state (slot 0 stays zero forever)
            h_st = state.tile([128, 2, SW], MM_DT)
            c_st = state.tile([128, 2, SW], C_DT)
            hsum = state.tile([128, 2, SW], F32)

            # (reps>1 repeats the computation via a HW loop, for slope-timing)
            import contextlib
            loop_cm = (tc.For_i(0, reps, 1) if reps > 1
                       else contextlib.nullcontext())
            with loop_cm:
                nc.vector.memset(h_st, 0.0)
                nc.gpsimd.memset(c_st, 0.0)
                nc.gpsimd.memset(hsum, 0.0)

                for d, tlo, nd, xoff in DIAGS:
                    N = nd * BL

                    for stlo, snd in _subwaves(d, tlo, nd):
                        Nw = snd * BL
                        ht, hs = stlo * BL, (stlo + 1) * BL
                        xc = xoff + (stlo - tlo) * BL

                        # x-GEMMs first, row-group packed 4-wide: distinct
                        # PSUM banks + distinct 32-row sub-arrays -> the PE
                        # runs them concurrently (and ahead, during the
                        # previous sub-wave's ACT/DVE tail)
                        ps = {}
                        for gname, _, _ in GATES:
                            ps[gname] = psg.tile([128, 2, 256], F32,
                                                 name="ps_" + gname, tag="ps")
                        # this order gives every consecutive 4 jobs distinct
                        # PSUM banks and (via idx%4) distinct row-groups
                        xjobs = [(g, m) for m in (0, 1) for g, _, _ in GATES]
                        gcol = dict((g, gc) for g, gc, _ in GATES)
                        for idx, (gname, m) in enumerate(xjobs):
                            packed = XPACK >= 1 and (m == 0 or XPACK == 1)
                            r = idx % 4 if packed else 0
                            mc = gcol[gname] + m * 128
                            # start=True clears has_written for the WHOLE
                            # bank -> only the first matmul touching each
                            # gate's (single-bank) tile may carry it
                            nc.tensor.matmul(
                                ps[gname][:, m, 0:Nw],
                                wih_s[32 * r:32 * r + 4, mc:mc + 128],
                                xs_all[32 * r:32 * r + 4, xc:xc + Nw],
                                start=(m == 0), stop=False,
                                tile_position=(32 * r, 0) if packed else None)

                        gt = {}
                        for gname, gc, fn in GATES:
                            for m in (0, 1):
                                mc = gc + m * 128
                                o = ps[gname][:, m, 0:Nw]
                                nc.tensor.matmul(
                                    o, wth_s[:, 0, mc:mc + 128],
                                    h_st[:, 0, ht:ht + Nw],
                                    start=False, stop=False)
                                nc.tensor.matmul(
                                    o, wth_s[:, 1, mc:mc + 128],
                                    h_st[:, 1, ht:ht + Nw],
                                    start=False, stop=False)
                                nc.tensor.matmul(
                                    o, wsh_s[:, 0, mc:mc + 128],
                                    h_st[:, 0, hs:hs + Nw],
                                    start=False, stop=False)
                                nc.tensor.matmul(
                                    o, wsh_s[:, 1, mc:mc + 128],
                                    h_st[:, 1, hs:hs + Nw],
                                    start=False, stop=True)
                            g = gatep.tile([128, 2, NSUB], GATE_DT, tag=gname)
                            nc.scalar.activation(out=g[:, :, 0:Nw],
                                                 in_=ps[gname][:, :, 0:Nw],
                                                 func=getattr(AF, fn))
                            gt[gname] = g

                        t1 = work.tile([128, 2, NSUB], GATE_DT, tag="t1")
                        nc.vector.tensor_mul(t1[:, :, 0:Nw],
                                             gt["i"][:, :, 0:Nw],
                                             gt["u"][:, :, 0:Nw])
                        t2 = work.tile([128, 2, NSUB], C_DT, tag="t2")
                        nc.vector.tensor_mul(t2[:, :, 0:Nw],
                                             gt["fs"][:, :, 0:Nw],
                                             c_st[:, :, hs:hs + Nw])
                        s12 = work.tile([128, 2, NSUB], C_DT, tag="s12")
                        nc.vector.tensor_add(s12[:, :, 0:Nw], t1[:, :, 0:Nw],
                                             t2[:, :, 0:Nw])
                        t3 = work.tile([128, 2, NSUB], C_DT, tag="t3")
                        nc.vector.tensor_mul(t3[:, :, 0:Nw],
                                             gt["ft"][:, :, 0:Nw],
                                             c_st[:, :, ht:ht + Nw])
                        nc.vector.tensor_add(c_st[:, :, hs:hs + Nw],
                                             s12[:, :, 0:Nw], t3[:, :, 0:Nw])
                        tcz = work.tile([128, 2, NSUB], GATE_DT, tag="tc")
                        nc.scalar.activation(out=tcz[:, :, 0:Nw],
                                             in_=c_st[:, :, hs:hs + Nw],
                                             func=AF.Tanh)
                        nc.vector.tensor_mul(h_st[:, :, hs:hs + Nw],
                                             gt["o"][:, :, 0:Nw],
                                             tcz[:, :, 0:Nw])

                    # one mean-pool accumulate for the whole diagonal
                    lo = (tlo + 1) * BL
                    nc.vector.tensor_add(hsum[:, :, lo:lo + N],
                                         hsum[:, :, lo:lo + N],
                                         h_st[:, :, lo:lo + N])

                # ---- mean-pool: fold slots 1..100 down onto slot 1
                cur = T
                while cur > 1:
                    if cur % 2 == 1:
                        last = BL + (cur - 1) * BL
                        nc.vector.tensor_add(hsum[:, :, BL:2 * BL],
                                             hsum[:, :, BL:2 * BL],
                                             hsum[:, :, last:last + BL])
                        cur -= 1
                    half = cur // 2
                    w = half * BL
                    nc.vector.tensor_add(hsum[:, :, BL:BL + w],
                                         hsum[:, :, BL:BL + w],
                                         hsum[:, :, BL + w:BL + 2 * w])
                    cur = half
                # F = hsum[:, :, BL:2*BL] (128, 2, 16) fp32; 1/(T*J) in wc

                # ---- classifier: out (BL, C), batch on partitions
                pc = pscls.tile([BL, 512], F32, tag="cls")
                lg = pc[:, 0:C]
                nc.tensor.matmul(lg, hsum[:, 0, BL:2 * BL], wc_s[:, 0, :],
                                 start=True, stop=False)
                nc.tensor.matmul(lg, hsum[:, 1, BL:2 * BL], wc_s[:, 1, :],
                                 start=False, stop=False)
                nc.tensor.matmul(lg, ones_s[:, :], bc_s[:, :],
                                 start=False, stop=True)

                # ---- log_softmax over free dim
                mx = work.tile([BL, 1], F32, tag="mx")
                nc.vector.reduce_max(out=mx, in_=lg,
                                     axis=mybir.AxisListType.X)
                nmx = work.tile([BL, 1], F32, tag="nmx")
                nc.scalar.mul(out=nmx, in_=mx, mul=-1.0)
                ex = work.tile([BL, C], F32, tag="ex")
                nc.scalar.activation(out=ex, in_=lg, func=AF.Exp, bias=nmx)
                sm = work.tile([BL, 1], F32, tag="sm")
                nc.vector.reduce_sum(out=sm, in_=ex,
                                     axis=mybir.AxisListType.X)
                lse = work.tile([BL, 1], F32, tag="lse")
                nc.scalar.activation(out=lse, in_=sm, func=AF.Ln)
                tot = work.tile([BL, 1], F32, tag="tot")
                nc.vector.tensor_add(tot, mx, lse)
                res = work.tile([BL, C], F32, tag="res")
                nc.vector.tensor_scalar(out=res, in0=lg, scalar1=tot,
                                        scalar2=None,
                                        op0=mybir.AluOpType.subtract)
                nc.sync.dma_start(out=out_d[:, :], in_=res)

    nc.compile()
    return nc


_NC = None


def _get_nc():
    global _NC
    if _NC is None:
        _NC = _build_nc()
    return _NC


def _pack_inputs(data, W_ih, W_th, W_sh, b, weight_c, bias_c):
    """Host-side prep: weights in lhsT layout, x in diagonal-major order."""
    data = np.asarray(data, np.float32)
    # lhsT for the x-GEMM: (I+1, 1280) = [W_ih.T; b] (bias via ones row in x),
    # replicated 4x for row-group packing
    wih = np.concatenate([np.asarray(W_ih, np.float32).T,
                          np.asarray(b, np.float32)[None, :]], 0).astype(MM_NP)
    wih = np.tile(wih, (4, 1))
    # lhsT for h-GEMMs: (128, chunk, 1280)
    wth = np.asarray(W_th, np.float32).T.reshape(2, 128, G5).transpose(1, 0, 2)
    wsh = np.asarray(W_sh, np.float32).T.reshape(2, 128, G5).transpose(1, 0, 2)
    wth = np.ascontiguousarray(wth).astype(MM_NP)
    wsh = np.ascontiguousarray(wsh).astype(MM_NP)
    # classifier: fold the 1/(T*J) mean into the weights
    wc = (np.asarray(weight_c, np.float32).T / (T * J)).reshape(2, 128, C)
    wc = np.ascontiguousarray(wc.transpose(1, 0, 2), np.float32)
    bc = np.asarray(bias_c, np.float32)[None, :]

    # x in diagonal-major order: cols (cell-in-diag, batch), rows (I..., ones)
    tt = np.concatenate([np.arange(max(0, d - (J - 1)), min(d, T - 1) + 1)
                         for d in range(T + J - 1)])
    jj = np.concatenate([d - np.arange(max(0, d - (J - 1)), min(d, T - 1) + 1)
                         for d in range(T + J - 1)])
    xc = data[tt, jj]                     # (2500, B, I)
    in_maps = []
    for k in range(NCORES):
        xk = xc[:, k * BL:(k + 1) * BL, :]          # (2500, BL, I)
        xk = xk.transpose(2, 0, 1).reshape(I, XCOLS)
        xdiag = np.concatenate([xk, np.ones((1, XCOLS), np.float32)], 0)
        xdiag = np.tile(xdiag, (4, 1))
        in_maps.append({
            "xdiag": np.ascontiguousarray(xdiag).astype(MM_NP),
            "wih": wih, "wth": wth, "wsh": wsh, "wc": wc, "bc": bc,
        })
    return in_maps


class _Runner:
    """Persistent jitted SPMD executable (run_bass_via_pjrt traces+jits on
    every call; this caches the jit and keeps inputs device-resident)."""

    def __init__(self, nc):
        import jax
        from jax.sharding import Mesh, PartitionSpec
        from jax.experimental.shard_map import shard_map
        from concourse import mybir as _mb
        from concourse.bass2jax import _bass_exec_p, install_neuronx_cc_hook

        install_neuronx_cc_hook()
        self.nc = nc
        in_names, out_names, out_avals, zero_outs = [], [], [], []
        for alloc in nc.m.functions[0].allocations:
            if not isinstance(alloc, _mb.MemoryLocationSet):
                continue
            name = alloc.memorylocations[0].name
            if alloc.kind == "ExternalInput":
                in_names.append(name)
            elif alloc.kind == "ExternalOutput":
                out_names.append(name)
                shape = tuple(alloc.tensor_shape)
                dtype = _mb.dt.np(alloc.dtype)
                out_avals.append(jax.core.ShapedArray(shape, dtype))
                zero_outs.append(np.zeros(shape, dtype))
        self.in_names, self.out_names = in_names, out_names
        n_params, n_outs = len(in_names), len(out_names)
        all_names = tuple(in_names + out_names)

        def _body(*args):
            return tuple(_bass_exec_p.bind(
                *args, out_avals=tuple(out_avals), in_names=all_names,
                out_names=tuple(out_names), lowering_input_output_aliases=(),
                sim_require_finite=True, sim_require_nnan=True, nc=nc))

        devices = jax.devices()[:NCORES]
        self.mesh = Mesh(np.asarray(devices), ("core",))
        in_specs = (PartitionSpec("core"),) * (n_params + n_outs)
        out_specs = (PartitionSpec("core"),) * n_outs
        self._jit = jax.jit(
            shard_map(_body, mesh=self.mesh, in_specs=in_specs,
                      out_specs=out_specs, check_rep=False),
            donate_argnums=tuple(range(n_params, n_params + n_outs)),
            keep_unused=True)
        self._zeros = zero_outs
        self._dev_in = None

    def put_inputs(self, in_maps):
        import jax
        from jax.sharding import NamedSharding, PartitionSpec
        sh = NamedSharding(self.mesh, PartitionSpec("core"))
        pid = (self.nc.partition_id_tensor.name
               if self.nc.partition_id_tensor else None)
        in_maps = [dict(m) for m in in_maps]
        for k, m in enumerate(in_maps):
            if pid is not None:
                m[pid] = np.array([[k]], dtype=np.uint32)
        self._dev_in = [
            jax.device_put(np.concatenate(
                [np.asarray(m[n]) for m in in_maps], 0), sh)
            for n in self.in_names]

    def run(self):
        zeros = [np.concatenate([z] * NCORES, 0) for z in self._zeros]
        outs = self._jit(*self._dev_in, *zeros)
        return [np.asarray(o) for o in outs]


_RUNNER = None


def _get_runner():
    global _RUNNER
    if _RUNNER is None:
        _RUNNER = _Runner(_get_nc())
    return _RUNNER


def run_on_device(in_maps):
    r = _get_runner()
    r.put_inputs(in_maps)
    out = r.run()[0]          # (8*BL, C) concat over cores
    return out.reshape(NCORES * BL, C)


def kernel(data, W_ih, W_th, W_sh, b, weight_c, bias_c, batch_size=None,
           **_ignored):
    in_maps = _pack_inputs(data, W_ih, W_th, W_sh, b, weight_c, bias_c)
    return run_on_device(in_maps)


if __name__ == "__main__":
    d = np.load(os.path.join(os.path.dirname(__file__), "inputs.npz"))
    out = kernel(d["data"], d["W_ih"], d["W_th"], d["W_sh"], d["b"],
                 d["weight_c"], d["bias_c"])
    exp = np.load(os.path.join(os.path.dirname(__file__), "oracle64.npy"))
    aerr = np.abs(out - exp).max()
    print("absmax err vs fp64 oracle:", aerr,
          " rel:", aerr / np.abs(exp).max())
